# revision 1
# baseline (speedup 1.0000x reference)
"""MixtureOfDepth Trainium2 Bass kernel (8-core SPMD).

Sharding: core c -> (batch b = c//4, rank r = c%4).
Each core: router matvec + exact top-511 selection (gpsimd kth_largest) +
compaction (gpsimd sparse_gather) + indirect-DMA token gather + pre-LN
attention block with RoPE (bf16 matmuls, f32 accum) replicated within the
batch group, and a rank-sliced quarter of the MLP (TP-4 over DFF).
Host combines: x3 = x2 + sum_r mlp_r; out[b, sel] = x3 * rw; passthrough
quarters are written by the device (DRAM->DRAM copy).
"""
import numpy as np

import concourse.bass as bass
import concourse.mybir as mybir
import concourse.tile as tile
from concourse import bacc, library_config
from concourse.bass import IndirectOffsetOnAxis
from concourse.bass_utils import run_bass_kernel_spmd

P = 128
B, S, D, H = 2, 4096, 1024, 16
HD = D // H           # 64
DFF = 4 * D           # 4096
DFF_SL = DFF // 4     # per-core MLP slice
M = 511               # selected tokens
MT = 512              # padded
NCH = S // P          # 32 token chunks
DG = D // P           # 8 feature groups
NEG = -1e9
EPS = 1e-5

FP = mybir.dt.float32
BF = mybir.dt.bfloat16
I32 = mybir.dt.int32
U32 = mybir.dt.uint32

AL = mybir.AluOpType
AF = mybir.ActivationFunctionType

_NC_CACHE = {}


def _build_nc():
    if "nc" in _NC_CACHE:
        return _NC_CACHE["nc"]
    nc = bacc.Bacc("TRN2", target_bir_lowering=False, debug=False)

    T = {}

    def din(name, shape, dt):
        T[name] = nc.dram_tensor(name, shape, dt, kind="ExternalInput")

    def dout(name, shape, dt):
        T[name] = nc.dram_tensor(name, shape, dt, kind="ExternalOutput")

    din("hid", [S, D], FP)
    din("hq", [S // 4, D], FP)
    din("wqd", [D, D], FP)
    din("wkd", [D, D], FP)
    din("wvd", [D, D], FP)
    din("wod", [D, D], FP)
    din("w1d", [D, DFF_SL], FP)
    din("w2d", [DFF_SL, D], FP)
    din("rw_rep", [P, D], FP)
    din("ln1g", [P, D], FP)
    din("ln1b", [P, D], FP)
    din("ln2g", [P, D], FP)
    din("ln2b", [P, D], FP)
    din("tok16_d", [16, 256], FP)
    din("onr_d", [1, P], FP)
    din("biota_d", [1, P], FP)
    din("onc_d", [P, 1], FP)
    din("idf_d", [P, P], FP)
    din("idb_d", [P, P], BF)
    din("tri_d", [P, MT], FP)
    din("cos_d", [S, HD // 2], FP)
    din("sin_d", [S, HD // 2], FP)

    dout("sel_lin", [MT, 1], FP)
    dout("rw_lin", [MT, 1], FP)
    dout("nfound", [1, 2], U32)
    dout("x2_out", [MT, D], FP)
    dout("mlp_out", [MT, D], FP)
    dout("outq", [S // 4, D], FP)

    with tile.TileContext(nc) as tc:
        _emit(nc, tc, T)
    nc.compile()
    _NC_CACHE["nc"] = nc
    return nc


def _emit(nc, tc, T):
    import contextlib
    with contextlib.ExitStack() as ctx:
        const = ctx.enter_context(tc.tile_pool(name="const", bufs=1))
        sb = ctx.enter_context(tc.tile_pool(name="sb", bufs=1))
        sb2 = ctx.enter_context(tc.tile_pool(name="sb2", bufs=2))
        stage = ctx.enter_context(tc.tile_pool(name="stage", bufs=3))
        wts = ctx.enter_context(tc.tile_pool(name="wts", bufs=2))
        # PSUM: mm(3) + mmb(1) + sc(2) + ctx(2) = 8 banks; rb shares mmb
        ppmm = ctx.enter_context(tc.tile_pool(name="ppmm", bufs=3, space="PSUM"))
        ppmb = ctx.enter_context(tc.tile_pool(name="ppmb", bufs=1, space="PSUM"))
        ppsc = ctx.enter_context(tc.tile_pool(name="ppsc", bufs=1, space="PSUM"))
        ppcx = ctx.enter_context(tc.tile_pool(name="ppcx", bufs=2, space="PSUM"))

        def cload(name, shape, dt):
            t = const.tile(shape, dt, tag=name, name=f"c_{name}")
            nc.sync.dma_start(t[:], T[name][:])
            return t

        tk16 = cload("tok16_d", [16, 256], FP)
        onr = cload("onr_d", [1, P], FP)
        biota = cload("biota_d", [1, P], FP)
        onc_like = cload("onc_d", [P, 1], FP)
        idf = cload("idf_d", [P, P], FP)
        idb = cload("idb_d", [P, P], BF)
        tri = cload("tri_d", [P, MT], FP)
        rwv = cload("rw_rep", [P, D], FP)
        l1g = cload("ln1g", [P, D], FP)
        l1b = cload("ln1b", [P, D], FP)
        l2g = cload("ln2g", [P, D], FP)
        l2b = cload("ln2b", [P, D], FP)

        # ---------- passthrough quarter copy (DRAM->DRAM) ----------
        for q in range(4):
            nc.sync.dma_start(T["outq"][q * 256:(q + 1) * 256, :],
                              T["hq"][q * 256:(q + 1) * 256, :])

        # ---------- router ----------
        w_sb = sb.tile([P, NCH], FP)
        for c in range(NCH):
            hchunk = stage.tile([P, D], FP, tag="stg")
            nc.sync.dma_start(hchunk[:], T["hid"][c * P:(c + 1) * P, :])
            jt = stage.tile([P, D], FP, tag="stg")
            nc.vector.tensor_mul(jt[:], hchunk[:], rwv[:])
            nc.vector.tensor_reduce(out=w_sb[:, c:c + 1], in_=jt[:],
                                    axis=mybir.AxisListType.X, op=AL.add)

        # ---------- exact threshold (512th largest) via bisection ----------
        # invariant: count(w > lo) >= 512 > count(w > hi); after 5 rounds of
        # 128-way narrowing the interval is < 1 ulp, so count(w > lo) == 511.
        lo = sb.tile([1, 1], FP)
        hi = sb.tile([1, 1], FP)
        nc.vector.memset(lo[:], -16.0)
        nc.vector.memset(hi[:], 16.0)
        stp = sb.tile([1, 1], FP)
        trow = sb.tile([1, P], FP)
        trep = sb.tile([P, P], FP)
        gcnt = sb.tile([P, P], FP)
        cntr = sb.tile([1, P], FP)
        mrow = sb.tile([1, P], FP)
        grow = sb.tile([1, P], I32)
        sc1 = sb.tile([1, 1], FP)
        for rnd in range(5):
            # thresholds t_j = lo + (j+1) * (hi - lo) / 129
            nc.vector.tensor_sub(out=stp[:], in0=hi[:], in1=lo[:])
            nc.vector.tensor_scalar_mul(stp[:], stp[:], 1.0 / 129.0)
            nc.vector.tensor_scalar(out=trow[:], in0=biota[:], scalar1=stp[:],
                                    scalar2=None, op0=AL.mult)
            nc.vector.tensor_scalar(out=trow[:], in0=trow[:], scalar1=lo[:],
                                    scalar2=None, op0=AL.add)
            tps = ppmm.tile([P, P], FP, tag="mm")
            nc.tensor.matmul(out=tps[:], lhsT=onr[:], rhs=trow[:],
                             start=True, stop=True)
            nc.scalar.copy(trep[:], tps[:])
            # per-(partition, threshold) counts over the 32 tokens
            gb = sb.tile([P, P, NCH], BF, tag="bisg")
            nc.vector.tensor_tensor(
                out=gb[:],
                in0=w_sb[:, None, :].to_broadcast([P, P, NCH]),
                in1=trep[:, :, None].to_broadcast([P, P, NCH]),
                op=AL.is_gt)
            nc.vector.tensor_reduce(out=gcnt[:], in_=gb[:],
                                    axis=mybir.AxisListType.X, op=AL.add)
            cps = ppmm.tile([1, P], FP, tag="mm")
            nc.tensor.matmul(out=cps[:], lhsT=onc_like[:], rhs=gcnt[:],
                             start=True, stop=True)
            nc.scalar.copy(cntr[:], cps[:])
            # lo <- max(lo, max{t_j : cnt_j >= 512})
            nc.vector.tensor_scalar(out=grow[:], in0=cntr[:], scalar1=510.5,
                                    scalar2=None, op0=AL.is_ge)
            nc.vector.memset(mrow[:], -1e30)
            nc.vector.copy_predicated(out=mrow[:], mask=grow[:], data=trow[:])
            nc.vector.tensor_reduce(out=sc1[:], in_=mrow[:],
                                    axis=mybir.AxisListType.X, op=AL.max)
            nc.vector.tensor_tensor(out=lo[:], in0=lo[:], in1=sc1[:], op=AL.max)
            # hi <- min(hi, min{t_j : cnt_j < 512})
            nc.vector.tensor_scalar(out=grow[:], in0=cntr[:], scalar1=510.5,
                                    scalar2=None, op0=AL.is_lt)
            nc.vector.memset(mrow[:], 1e30)
            nc.vector.copy_predicated(out=mrow[:], mask=grow[:], data=trow[:])
            nc.vector.tensor_reduce(out=sc1[:], in_=mrow[:],
                                    axis=mybir.AxisListType.X, op=AL.min)
            nc.vector.tensor_tensor(out=hi[:], in0=hi[:], in1=sc1[:], op=AL.min)
        thr_ps = ppmm.tile([P, 1], FP, tag="mm")
        nc.tensor.matmul(out=thr_ps[:], lhsT=onr[:], rhs=lo[:],
                         start=True, stop=True)
        thr_bc = sb.tile([P, 1], FP)
        nc.scalar.copy(thr_bc[:], thr_ps[:])

        # ---------- compaction via sparse_gather (16-wrap token order) ----------
        t1ps = ppmm.tile([NCH, P], FP, tag="mm")
        nc.tensor.transpose(out=t1ps[:], in_=w_sb[:], identity=idf[:])
        t1 = sb.tile([NCH, P], FP)
        nc.scalar.copy(t1[:], t1ps[:])
        w16 = sb.tile([16, 256], FP)
        w16v = w16[:].rearrange("p (c q) -> p c q", q=8)
        for q in range(8):
            tq = ppmm.tile([16, NCH], FP, tag="mm")
            nc.tensor.transpose(out=tq[:], in_=t1[:, 16 * q:16 * (q + 1)],
                                identity=idf[0:NCH, 0:NCH])
            nc.scalar.copy(w16v[:, :, q], tq[:])

        mask16 = sb.tile([16, 256], FP)
        nc.vector.tensor_scalar(out=mask16[:], in0=w16[:], scalar1=thr_bc[0:16, :],
                                scalar2=None, op0=AL.is_gt)
        selv = sb.tile([16, 256], FP)
        nc.vector.tensor_mul(selv[:], tk16[:], mask16[:])
        nc.vector.tensor_scalar(out=selv[:], in0=selv[:], scalar1=1.0,
                                scalar2=None, op0=AL.subtract)
        m16i = sb.tile([16, 256], I32)
        nc.vector.tensor_copy(m16i[:], mask16[:])
        rwv16 = sb.tile([16, 256], FP)
        nc.vector.memset(rwv16[:], -1e30)
        nc.vector.copy_predicated(out=rwv16[:], mask=m16i[:], data=w16[:])

        sel16 = sb.tile([16, 32], FP)
        rw16 = sb.tile([16, 32], FP)
        nf = sb.tile([1, 2], U32)
        with tc.tile_critical():
            nc.gpsimd.load_library(library_config.sparse_gather)
            nc.gpsimd.sparse_gather(sel16[:], selv[:], num_found=nf[0:1, 0:1])
            nc.gpsimd.sparse_gather(rw16[:], rwv16[:], num_found=nf[0:1, 1:2])
        nc.sync.dma_start(T["nfound"][:], nf[:])
        nc.sync.dma_start(T["sel_lin"][:].rearrange("(f p) x -> p (f x)", p=16),
                          sel16[:])
        nc.sync.dma_start(T["rw_lin"][:].rearrange("(f p) x -> p (f x)", p=16),
                          rw16[:])

        sel_f = sb.tile([P, 4], FP)
        nc.sync.dma_start(sel_f[:],
                          T["sel_lin"][:].rearrange("(g p) x -> p (g x)", p=P))
        sel_sb = sb.tile([P, 4], I32)
        nc.vector.tensor_copy(sel_sb[:], sel_f[:])
        nc.vector.tensor_scalar(out=sel_sb[:], in0=sel_sb[:], scalar1=S - 1,
                                scalar2=None, op0=AL.min)
        nc.vector.tensor_scalar(out=sel_sb[:], in0=sel_sb[:], scalar1=0,
                                scalar2=None, op0=AL.max)

        # ---------- gathers ----------
        x1 = sb.tile([P, 4, D], FP, tag="big")
        cos_g = sb.tile([P, 4, HD // 2], FP)
        sin_g = sb.tile([P, 4, HD // 2], FP)
        for g in range(4):
            io = IndirectOffsetOnAxis(ap=sel_sb[:, g:g + 1], axis=0)
            nc.gpsimd.indirect_dma_start(out=x1[:, g, :], out_offset=None,
                                         in_=T["hid"][:], in_offset=io)
            nc.gpsimd.indirect_dma_start(out=cos_g[:, g, :], out_offset=None,
                                         in_=T["cos_d"][:], in_offset=io)
            nc.gpsimd.indirect_dma_start(out=sin_g[:, g, :], out_offset=None,
                                         in_=T["sin_d"][:], in_offset=io)

        # cos/sin transposed and replicated on all four 32-partition blocks
        cosT = sb.tile([P, MT], FP)
        sinT = sb.tile([P, MT], FP)
        for g in range(4):
            cps = ppmm.tile([32, P], FP, tag="mm")
            nc.tensor.transpose(out=cps[:], in_=cos_g[:, g, :], identity=idf[:])
            for bb in range(4):
                nc.scalar.copy(cosT[32 * bb:32 * (bb + 1), g * P:(g + 1) * P], cps[:])
            sps = ppmm.tile([32, P], FP, tag="mm")
            nc.tensor.transpose(out=sps[:], in_=sin_g[:, g, :], identity=idf[:])
            for bb in range(4):
                nc.scalar.copy(sinT[32 * bb:32 * (bb + 1), g * P:(g + 1) * P], sps[:])
        cosq = sb.tile([P, MT], FP)
        sinq = sb.tile([P, MT], FP)
        sc = 1.0 / np.sqrt(HD)
        nc.vector.tensor_scalar_mul(cosq[:], cosT[:], sc)
        nc.vector.tensor_scalar_mul(sinq[:], sinT[:], sc)

        # ---------- LN1 ----------
        h_bf = sb.tile([P, 4, D], BF, tag="actN")
        _layernorm(nc, sb, stage, x1, h_bf, l1g, l1b)

        # ---------- transpose h ----------
        hT = sb.tile([P, DG, MT], BF, tag="actT")
        _transpose_nat_to_T(nc, ppmb, h_bf, hT, idb)

        def wload(dram, cols):
            wt = wts.tile([P, DG, cols], BF, tag="w")
            for dg in range(DG):
                st = stage.tile([P, cols], FP, tag="stg")
                nc.sync.dma_start(st[:], dram[dg * P:(dg + 1) * P, :])
                nc.scalar.copy(wt[:, dg, :], st[:])
            return wt

        # ---------- QKV (transposed) + RoPE in place ----------
        wq_bf = wload(T["wqd"], D)
        qT = sb.tile([P, DG, MT], BF)
        _proj_T(nc, ppmm, wq_bf, hT, qT)
        wk_bf = wload(T["wkd"], D)
        kT = sb.tile([P, DG, MT], BF)
        _proj_T(nc, ppmm, wk_bf, hT, kT)
        _rope(nc, sb, qT, cosq, sinq)
        _rope(nc, sb, kT, cosT, sinT)

        # ---------- V natural + interleaved ones ----------
        wv_bf = wload(T["wvd"], D)
        vN2 = sb.tile([P, 4, H * (HD + 1)], BF)
        for tc_ in range(4):
            for half in range(2):
                vp = ppmm.tile([P, MT], FP, tag="mm")
                for dg in range(DG):
                    nc.tensor.matmul(
                        out=vp[:], lhsT=hT[:, dg, tc_ * P:(tc_ + 1) * P],
                        rhs=wv_bf[:, dg, half * 512:(half + 1) * 512],
                        start=(dg == 0), stop=(dg == DG - 1))
                dst = vN2[:, tc_, :].rearrange("p (h e) -> p h e", e=HD + 1)
                nc.scalar.copy(dst[:, half * 8:(half + 1) * 8, 0:HD],
                               vp[:].rearrange("p (h e) -> p h e", e=HD))
        nc.vector.memset(
            vN2[:, :, :].rearrange("p g (h e) -> p g h e", e=HD + 1)[:, :, :, HD:HD + 1],
            1.0)

        # ---------- attention (waves of 2 heads) ----------
        ctxT = sb.tile([P, DG, MT], BF)
        for wv_ in range(8):
            scps = ppsc.tile([P, 2, MT], FP, tag="sc")
            expb = sb2.tile([P, 2, MT], BF, tag="expb")
            ctps = [ppcx.tile([HD + 1, MT], FP, tag="cx", name=f"ctps{wv_}_{j}")
                    for j in range(2)]
            for kt in range(4):
                qt0 = P * kt
                qtw = MT - qt0
                for j in range(2):
                    h = 2 * wv_ + j
                    m, o = h // 2, HD * (h % 2)
                    nc.tensor.matmul(
                        out=scps[:, j, qt0:MT],
                        lhsT=kT[o:o + HD, m, kt * P:(kt + 1) * P],
                        rhs=qT[o:o + HD, m, qt0:MT],
                        start=True, stop=True)
                nc.vector.tensor_tensor(
                    out=scps[:, :, qt0:MT], in0=scps[:, :, qt0:MT],
                    in1=tri[:, None, 0:qtw].to_broadcast([P, 2, qtw]),
                    op=AL.add)
                nc.scalar.activation(expb[:, :, qt0:MT], scps[:, :, qt0:MT], AF.Exp)
                for j in range(2):
                    h = 2 * wv_ + j
                    nc.tensor.matmul(
                        out=ctps[j][:, qt0:MT],
                        lhsT=vN2[:, kt, h * (HD + 1):(h + 1) * (HD + 1)],
                        rhs=expb[:, j, qt0:MT],
                        start=(kt == 0), stop=(kt == 3))
            for j in range(2):
                h = 2 * wv_ + j
                m, o = h // 2, HD * (h % 2)
                rec = sb2.tile([1, MT], FP, tag="rec")
                nc.vector.reciprocal(rec[:], ctps[j][HD:HD + 1, :])
                rbps = ppmb.tile([HD, MT], FP, tag="mmb")
                nc.tensor.matmul(out=rbps[:], lhsT=onr[0:1, 0:HD], rhs=rec[:],
                                 start=True, stop=True)
                rbsb = sb2.tile([HD, MT], FP, tag="rbsb")
                nc.scalar.copy(rbsb[:], rbps[:])
                nc.vector.tensor_tensor(out=ctxT[o:o + HD, m, :],
                                        in0=ctps[j][0:HD, :], in1=rbsb[:],
                                        op=AL.mult)

        # ---------- Wo + residual ----------
        wo_bf = wload(T["wod"], D)
        x2 = sb.tile([P, 4, D], FP)
        for tc_ in range(4):
            for half in range(2):
                wops = ppmm.tile([P, MT], FP, tag="mm")
                for hg in range(DG):
                    nc.tensor.matmul(
                        out=wops[:], lhsT=ctxT[:, hg, tc_ * P:(tc_ + 1) * P],
                        rhs=wo_bf[:, hg, half * 512:(half + 1) * 512],
                        start=(hg == 0), stop=(hg == DG - 1))
                nc.vector.tensor_add(
                    out=x2[:, tc_, half * 512:(half + 1) * 512],
                    in0=x1[:, tc_, half * 512:(half + 1) * 512], in1=wops[:])
        nc.sync.dma_start(T["x2_out"][:].rearrange("(g p) d -> p g d", p=P), x2[:])

        # ---------- LN2 + transpose ----------
        h2_bf = sb.tile([P, 4, D], BF, tag="actN")
        _layernorm(nc, sb, stage, x2, h2_bf, l2g, l2b)
        h2T = sb.tile([P, DG, MT], BF, tag="actT")
        _transpose_nat_to_T(nc, ppmb, h2_bf, h2T, idb)

        # ---------- MLP slice ----------
        w1_bf = wload(T["w1d"], DFF_SL)
        w2_bf = wload(T["w2d"], D)
        geluT = sb.tile([P, DG, MT], BF, tag="big")
        for fm in range(DG):
            h1ps = ppmm.tile([P, MT], FP, tag="mm")
            for dg in range(DG):
                nc.tensor.matmul(
                    out=h1ps[:], lhsT=w1_bf[:, dg, fm * P:(fm + 1) * P],
                    rhs=h2T[:, dg, :],
                    start=(dg == 0), stop=(dg == DG - 1))
            nc.scalar.activation(geluT[:, fm, :], h1ps[:], AF.Gelu_apprx_tanh)
        for tc_ in range(4):
            for half in range(2):
                m2ps = ppmm.tile([P, MT], FP, tag="mm")
                for fg in range(DG):
                    nc.tensor.matmul(
                        out=m2ps[:], lhsT=geluT[:, fg, tc_ * P:(tc_ + 1) * P],
                        rhs=w2_bf[:, fg, half * 512:(half + 1) * 512],
                        start=(fg == 0), stop=(fg == DG - 1))
                mst = sb2.tile([P, MT], FP, tag="mst")
                nc.scalar.copy(mst[:], m2ps[:])
                nc.sync.dma_start(
                    T["mlp_out"][:].rearrange("(g p) d -> p g d", p=P)[
                        :, tc_, half * 512:(half + 1) * 512],
                    mst[:])


def _layernorm(nc, sb, stage, x, out_bf, g_rep, b_rep):
    """x [128, 4, D] f32 -> out_bf [128, 4, D] bf16 = LN(x)*g + b."""
    stat = sb.tile([P, 4], FP, tag="lnsum")
    nc.vector.tensor_reduce(out=stat[:], in_=x[:], axis=mybir.AxisListType.X,
                            op=AL.add)
    mu = sb.tile([P, 4], FP, tag="lnmu")
    nc.vector.tensor_scalar_mul(mu[:], stat[:], 1.0 / D)
    var = sb.tile([P, 4], FP, tag="lnvar")
    for g in range(4):
        xc = stage.tile([P, D], FP, tag="stg")
        nc.vector.tensor_scalar(out=xc[:], in0=x[:, g, :],
                                scalar1=mu[:, g:g + 1], scalar2=None,
                                op0=AL.subtract)
        jt = stage.tile([P, D], FP, tag="stg")
        nc.vector.tensor_mul(jt[:], xc[:], xc[:])
        nc.vector.tensor_reduce(out=var[:, g:g + 1], in_=jt[:],
                                axis=mybir.AxisListType.X, op=AL.add)
    sd = sb.tile([P, 4], FP, tag="lnsd")
    nc.vector.tensor_scalar(out=sd[:], in0=var[:], scalar1=1.0 / D, scalar2=EPS,
                            op0=AL.mult, op1=AL.add)
    nc.scalar.sqrt(sd[:], sd[:])
    rstd = sb.tile([P, 4], FP, tag="lnrstd")
    nc.vector.reciprocal(rstd[:], sd[:])
    for g in range(4):
        xc = stage.tile([P, D], FP, tag="stg")
        nc.vector.tensor_scalar(out=xc[:], in0=x[:, g, :],
                                scalar1=mu[:, g:g + 1], scalar2=None,
                                op0=AL.subtract)
        nc.vector.tensor_scalar(out=xc[:], in0=xc[:],
                                scalar1=rstd[:, g:g + 1], scalar2=None,
                                op0=AL.mult)
        nc.vector.tensor_mul(out=xc[:], in0=xc[:], in1=g_rep[:])
        nc.vector.tensor_tensor(out=out_bf[:, g, :], in0=xc[:],
                                in1=b_rep[:], op=AL.add)


def _transpose_nat_to_T(nc, ppmb, nat_bf, outT, idb):
    """[128(tok), 4, D] bf16 -> [128(d), 8, 512(tok)] bf16 via PE."""
    for g in range(4):
        for m in range(DG):
            tp = ppmb.tile([P, P], BF, tag="mmb")
            nc.tensor.transpose(out=tp[:], in_=nat_bf[:, g, m * P:(m + 1) * P],
                                identity=idb[:])
            nc.scalar.copy(outT[:, m, g * P:(g + 1) * P], tp[:])


def _proj_T(nc, ppmm, w_bf, hT, outT):
    """outT[128, 8, 512] = (h @ W)^T; W loaded [128, 8, D]."""
    for m in range(DG):
        pp = ppmm.tile([P, MT], FP, tag="mm")
        for dg in range(DG):
            nc.tensor.matmul(out=pp[:], lhsT=w_bf[:, dg, m * P:(m + 1) * P],
                             rhs=hT[:, dg, :],
                             start=(dg == 0), stop=(dg == DG - 1))
        nc.scalar.copy(outT[:, m, :], pp[:])


def _rope(nc, sbp, xT, cosv, sinv):
    """In-place RoPE on transposed q/k [128, 8, 512]; pairs (p, p+32)/64-block.

    Two half-passes over the middle dim to bound temp size.
    """
    for half in range(2):
        gs = slice(half * 4, half * 4 + 4)
        for base in (0, 64):
            cb = cosv[base:base + 32, None, :].to_broadcast([32, 4, MT])
            sbr = sinv[base:base + 32, None, :].to_broadcast([32, 4, MT])
            cb2 = cosv[base + 32:base + 64, None, :].to_broadcast([32, 4, MT])
            sb2r = sinv[base + 32:base + 64, None, :].to_broadcast([32, 4, MT])
            a1 = xT[base:base + 32, gs, :]
            a2 = xT[base + 32:base + 64, gs, :]
            t1c = sbp.tile([32, 4, MT], BF, tag="rp1")
            t1s = sbp.tile([32, 4, MT], BF, tag="rp2")
            t2s = sbp.tile([32, 4, MT], BF, tag="rp3")
            nc.vector.tensor_tensor(out=t1c[:], in0=a1, in1=cb, op=AL.mult)
            nc.vector.tensor_tensor(out=t1s[:], in0=a1, in1=sbr, op=AL.mult)
            nc.vector.tensor_tensor(out=t2s[:], in0=a2, in1=sb2r, op=AL.mult)
            # a1 <- a1*cos - a2*sin  (t1c base 0/64 vs t2s base 0: temps all base 0)
            nc.vector.tensor_tensor(out=a1, in0=t1c[:], in1=t2s[:],
                                    op=AL.subtract)
            # a2 <- a1_old*sin + a2*cos
            nc.vector.tensor_tensor(out=t1c[:], in0=a2, in1=cb2, op=AL.mult)
            nc.vector.tensor_tensor(out=a2, in0=t1s[:], in1=t1c[:], op=AL.add)


# ======================= host side =======================

def _consts():
    import ml_dtypes
    c = {}
    c["tok16_d"] = (np.arange(S, dtype=np.float32) + 1).reshape(256, 16).T.copy()
    c["onr_d"] = np.ones((1, P), np.float32)
    c["biota_d"] = (np.arange(P, dtype=np.float32) + 1).reshape(1, P)
    c["onc_d"] = np.ones((P, 1), np.float32)
    c["idf_d"] = np.eye(P, dtype=np.float32)
    c["idb_d"] = np.eye(P).astype(ml_dtypes.bfloat16)
    p_ = np.arange(P)[:, None]
    f_ = np.arange(MT)[None, :]
    c["tri_d"] = np.where(p_ <= f_, 0.0, NEG).astype(np.float32)
    inv = (1.0 / (10000.0 ** (np.arange(0, HD, 2, dtype=np.float32) / HD)))
    ang = np.arange(S, dtype=np.float32)[:, None] * inv[None, :]
    c["cos_d"] = np.cos(ang).astype(np.float32)
    c["sin_d"] = np.sin(ang).astype(np.float32)
    return c


def kernel(hidden_states, attention_mask, position_ids, router_w,
           Wq, Wk, Wv, Wo, W1, W2, ln1_g, ln1_b, ln2_g, ln2_b):
    hidden_states = np.ascontiguousarray(np.asarray(hidden_states, np.float32))
    router_w = np.asarray(router_w, np.float32)
    nc = _build_nc()
    c = _consts()
    rep = lambda v: np.ascontiguousarray(
        np.broadcast_to(np.asarray(v, np.float32)[None, :], (P, D)))
    shared = {
        "wqd": np.ascontiguousarray(np.asarray(Wq, np.float32)),
        "wkd": np.ascontiguousarray(np.asarray(Wk, np.float32)),
        "wvd": np.ascontiguousarray(np.asarray(Wv, np.float32)),
        "wod": np.ascontiguousarray(np.asarray(Wo, np.float32)),
        "rw_rep": np.ascontiguousarray(
            np.broadcast_to(router_w[:, 0][None, :], (P, D))),
        "ln1g": rep(ln1_g), "ln1b": rep(ln1_b),
        "ln2g": rep(ln2_g), "ln2b": rep(ln2_b),
        **c,
    }
    W1 = np.asarray(W1, np.float32)
    W2 = np.asarray(W2, np.float32)
    in_maps = []
    for core in range(8):
        b, r = core // 4, core % 4
        m = dict(shared)
        m["hid"] = hidden_states[b]
        m["hq"] = np.ascontiguousarray(hidden_states[b, r * 1024:(r + 1) * 1024])
        m["w1d"] = np.ascontiguousarray(W1[:, r * DFF_SL:(r + 1) * DFF_SL])
        m["w2d"] = np.ascontiguousarray(W2[r * DFF_SL:(r + 1) * DFF_SL, :])
        in_maps.append(m)

    res = run_bass_kernel_spmd(nc, in_maps, core_ids=list(range(8)))

    out = np.empty_like(hidden_states)
    for b in range(2):
        g0 = 4 * b
        for r in range(4):
            out[b, r * 1024:(r + 1) * 1024] = res.results[g0 + r]["outq"]
        nf = res.results[g0]["nfound"]
        assert nf[0, 0] == M and nf[0, 1] == M, f"compaction found {nf}"
        sel = res.results[g0]["sel_lin"][:M, 0].astype(np.int64)
        rw = res.results[g0]["rw_lin"][:M, 0]
        x2 = res.results[g0]["x2_out"][:M]
        mlp = sum(res.results[g0 + r]["mlp_out"][:M] for r in range(4))
        x3 = x2 + mlp
        out[b, sel] = x3 * rw[:, None]
    return out



# revision 4
# speedup vs baseline: 18.5446x; 18.5446x over previous
"""MixtureOfDepth Trainium2 Bass kernel (2-core SPMD, host-side routing).

Router matvec, top-k threshold selection, token gather and scatter are
per-batch-row independent and tiny, so they run on the host with the exact
same jax CPU ops as the reference (bit-identical selection). The device
(core b = batch b) runs only the dense transformer block on the 511
selected tokens: pre-LN attention with RoPE (bf16 matmuls, f32 accum) and
the full-DFF MLP with W1/W2 streamed in 1024-column chunks. Weights ship
pre-cast to bf16; passthrough rows never leave the host.
"""
import numpy as np

import concourse.bass as bass
import concourse.mybir as mybir
import concourse.tile as tile
from concourse import bacc
from concourse.bass_utils import run_bass_kernel_spmd

P = 128
B, S, D, H = 2, 4096, 1024, 16
HD = D // H           # 64
DFF = 4 * D           # 4096
M = 511               # selected tokens
MT = 512              # padded
DG = D // P           # 8 feature groups
NF = DFF // 1024      # MLP chunks
NEG = -1e9
EPS = 1e-5

FP = mybir.dt.float32
BF = mybir.dt.bfloat16

AL = mybir.AluOpType
AF = mybir.ActivationFunctionType

_NC_CACHE = {}


def _build_nc():
    if "nc" in _NC_CACHE:
        return _NC_CACHE["nc"]
    nc = bacc.Bacc("TRN2", target_bir_lowering=False, debug=False)

    T = {}

    def din(name, shape, dt):
        T[name] = nc.dram_tensor(name, shape, dt, kind="ExternalInput")

    def dout(name, shape, dt):
        T[name] = nc.dram_tensor(name, shape, dt, kind="ExternalOutput")

    din("tok", [MT, D], FP)
    din("cosT_d", [P, MT], FP)
    din("sinT_d", [P, MT], FP)
    din("wqd", [D, D], BF)
    din("wkd", [D, D], BF)
    din("wvd", [D, D], BF)
    din("wod", [D, D], BF)
    din("w1d", [D, DFF], BF)
    din("w2d", [DFF, D], BF)
    din("ln1g", [P, D], FP)
    din("ln1b", [P, D], FP)
    din("ln2g", [P, D], FP)
    din("ln2b", [P, D], FP)
    din("onr_d", [1, P], FP)
    din("idb_d", [P, P], BF)
    din("tri_d", [P, MT], FP)

    dout("x3o", [MT, D], FP)

    with tile.TileContext(nc) as tc:
        _emit(nc, tc, T)
    nc.compile()
    _NC_CACHE["nc"] = nc
    return nc


def _emit(nc, tc, T):
    import contextlib
    with contextlib.ExitStack() as ctx:
        const = ctx.enter_context(tc.tile_pool(name="const", bufs=1))
        sb = ctx.enter_context(tc.tile_pool(name="sb", bufs=1))
        sb2 = ctx.enter_context(tc.tile_pool(name="sb2", bufs=2))
        stage = ctx.enter_context(tc.tile_pool(name="stage", bufs=3))
        wts = ctx.enter_context(tc.tile_pool(name="wts", bufs=2))
        # PSUM: mm(3) + mmb(1) + sc(2) + ctx(2) = 8 banks
        ppmm = ctx.enter_context(tc.tile_pool(name="ppmm", bufs=3, space="PSUM"))
        ppmb = ctx.enter_context(tc.tile_pool(name="ppmb", bufs=1, space="PSUM"))
        ppsc = ctx.enter_context(tc.tile_pool(name="ppsc", bufs=1, space="PSUM"))
        ppcx = ctx.enter_context(tc.tile_pool(name="ppcx", bufs=2, space="PSUM"))

        def cload(name, shape, dt):
            t = const.tile(shape, dt, tag=name, name=f"c_{name}")
            nc.sync.dma_start(t[:], T[name][:])
            return t

        onr = cload("onr_d", [1, P], FP)
        idb = cload("idb_d", [P, P], BF)
        tri = cload("tri_d", [P, MT], FP)
        cosT = cload("cosT_d", [P, MT], FP)
        sinT = cload("sinT_d", [P, MT], FP)
        l1g = cload("ln1g", [P, D], FP)
        l1b = cload("ln1b", [P, D], FP)
        l2g = cload("ln2g", [P, D], FP)
        l2b = cload("ln2b", [P, D], FP)

        # ---------- selected tokens (natural layout, t = g*128 + p) ----------
        x1 = sb.tile([P, 4, D], FP, tag="x1")
        nc.sync.dma_start(x1[:], T["tok"][:].rearrange("(g p) d -> p g d", p=P))

        # ---------- LN1 ----------
        h_bf = sb.tile([P, 4, D], BF, tag="actN")
        _layernorm(nc, sb, stage, x1, h_bf, l1g, l1b)

        # ---------- transpose h ----------
        hT = sb.tile([P, DG, MT], BF, tag="actT")
        _transpose_nat_to_T(nc, ppmb, h_bf, hT, idb)

        def wload(dram, col0, cols):
            wt = wts.tile([P, DG, cols], BF, tag="w")
            for dg in range(DG):
                nc.sync.dma_start(wt[:, dg, :],
                                  dram[dg * P:(dg + 1) * P, col0:col0 + cols])
            return wt

        # ---------- QKV (transposed) + RoPE in place ----------
        # Wq is pre-scaled by 1/sqrt(HD) on the host, so q/k RoPE share cos/sin.
        wq_bf = wload(T["wqd"], 0, D)
        qT = sb.tile([P, DG, MT], BF, tag="qT")
        _proj_T(nc, ppmm, wq_bf, hT, qT)
        wk_bf = wload(T["wkd"], 0, D)
        kT = sb.tile([P, DG, MT], BF, tag="kT")
        _proj_T(nc, ppmm, wk_bf, hT, kT)
        _rope(nc, sb, qT, cosT, sinT)
        _rope(nc, sb, kT, cosT, sinT)

        # ---------- V natural + interleaved ones ----------
        wv_bf = wload(T["wvd"], 0, D)
        vN2 = sb.tile([P, 4, H * (HD + 1)], BF, tag="vN2")
        for tc_ in range(4):
            for half in range(2):
                vp = ppmm.tile([P, MT], FP, tag="mm")
                for dg in range(DG):
                    nc.tensor.matmul(
                        out=vp[:], lhsT=hT[:, dg, tc_ * P:(tc_ + 1) * P],
                        rhs=wv_bf[:, dg, half * 512:(half + 1) * 512],
                        start=(dg == 0), stop=(dg == DG - 1))
                dst = vN2[:, tc_, :].rearrange("p (h e) -> p h e", e=HD + 1)
                nc.scalar.copy(dst[:, half * 8:(half + 1) * 8, 0:HD],
                               vp[:].rearrange("p (h e) -> p h e", e=HD))
        nc.vector.memset(
            vN2[:, :, :].rearrange("p g (h e) -> p g h e", e=HD + 1)[:, :, :, HD:HD + 1],
            1.0)

        # ---------- attention (waves of 2 heads) ----------
        ctxT = sb.tile([P, DG, MT], BF, tag="ctxT")
        for wv_ in range(8):
            scps = ppsc.tile([P, 2, MT], FP, tag="sc")
            expb = sb2.tile([P, 2, MT], BF, tag="expb")
            ctps = [ppcx.tile([HD + 1, MT], FP, tag="cx", name=f"ctps{wv_}_{j}")
                    for j in range(2)]
            for kt in range(4):
                qt0 = P * kt
                qtw = MT - qt0
                for j in range(2):
                    h = 2 * wv_ + j
                    m, o = h // 2, HD * (h % 2)
                    nc.tensor.matmul(
                        out=scps[:, j, qt0:MT],
                        lhsT=kT[o:o + HD, m, kt * P:(kt + 1) * P],
                        rhs=qT[o:o + HD, m, qt0:MT],
                        start=True, stop=True)
                nc.vector.tensor_tensor(
                    out=scps[:, :, qt0:MT], in0=scps[:, :, qt0:MT],
                    in1=tri[:, None, 0:qtw].to_broadcast([P, 2, qtw]),
                    op=AL.add)
                nc.scalar.activation(expb[:, :, qt0:MT], scps[:, :, qt0:MT], AF.Exp)
                for j in range(2):
                    h = 2 * wv_ + j
                    nc.tensor.matmul(
                        out=ctps[j][:, qt0:MT],
                        lhsT=vN2[:, kt, h * (HD + 1):(h + 1) * (HD + 1)],
                        rhs=expb[:, j, qt0:MT],
                        start=(kt == 0), stop=(kt == 3))
            for j in range(2):
                h = 2 * wv_ + j
                m, o = h // 2, HD * (h % 2)
                rec = sb2.tile([1, MT], FP, tag="rec")
                nc.vector.reciprocal(rec[:], ctps[j][HD:HD + 1, :])
                rbps = ppmb.tile([HD, MT], FP, tag="mmb")
                nc.tensor.matmul(out=rbps[:], lhsT=onr[0:1, 0:HD], rhs=rec[:],
                                 start=True, stop=True)
                rbsb = sb2.tile([HD, MT], FP, tag="rbsb")
                nc.scalar.copy(rbsb[:], rbps[:])
                nc.vector.tensor_tensor(out=ctxT[o:o + HD, m, :],
                                        in0=ctps[j][0:HD, :], in1=rbsb[:],
                                        op=AL.mult)

        # ---------- Wo + residual (x2 accumulated in place into x1) ----------
        wo_bf = wload(T["wod"], 0, D)
        for tc_ in range(4):
            for half in range(2):
                wops = ppmm.tile([P, MT], FP, tag="mm")
                for hg in range(DG):
                    nc.tensor.matmul(
                        out=wops[:], lhsT=ctxT[:, hg, tc_ * P:(tc_ + 1) * P],
                        rhs=wo_bf[:, hg, half * 512:(half + 1) * 512],
                        start=(hg == 0), stop=(hg == DG - 1))
                nc.vector.tensor_add(
                    out=x1[:, tc_, half * 512:(half + 1) * 512],
                    in0=x1[:, tc_, half * 512:(half + 1) * 512], in1=wops[:])

        # ---------- LN2 + transpose ----------
        h2_bf = sb.tile([P, 4, D], BF, tag="actN2")
        _layernorm(nc, sb, stage, x1, h2_bf, l2g, l2b)
        h2T = sb.tile([P, DG, MT], BF, tag="actT2")
        _transpose_nat_to_T(nc, ppmb, h2_bf, h2T, idb)

        # ---------- full-DFF MLP, streamed in NF chunks of 1024 ----------
        for c in range(NF):
            w1c = wload(T["w1d"], c * 1024, 1024)
            geluT = sb2.tile([P, DG, MT], BF, tag="gel")
            for fm in range(DG):
                h1ps = ppmm.tile([P, MT], FP, tag="mm")
                for dg in range(DG):
                    nc.tensor.matmul(
                        out=h1ps[:], lhsT=w1c[:, dg, fm * P:(fm + 1) * P],
                        rhs=h2T[:, dg, :],
                        start=(dg == 0), stop=(dg == DG - 1))
                nc.scalar.activation(geluT[:, fm, :], h1ps[:], AF.Gelu_apprx_tanh)
            w2c = wts.tile([P, DG, D], BF, tag="w")
            for dg in range(DG):
                nc.sync.dma_start(
                    w2c[:, dg, :],
                    T["w2d"][c * 1024 + dg * P:c * 1024 + (dg + 1) * P, :])
            for tc_ in range(4):
                for half in range(2):
                    m2ps = ppmm.tile([P, MT], FP, tag="mm")
                    for fg in range(DG):
                        nc.tensor.matmul(
                            out=m2ps[:], lhsT=geluT[:, fg, tc_ * P:(tc_ + 1) * P],
                            rhs=w2c[:, fg, half * 512:(half + 1) * 512],
                            start=(fg == 0), stop=(fg == DG - 1))
                    nc.vector.tensor_add(
                        out=x1[:, tc_, half * 512:(half + 1) * 512],
                        in0=x1[:, tc_, half * 512:(half + 1) * 512],
                        in1=m2ps[:])

        nc.sync.dma_start(T["x3o"][:].rearrange("(g p) d -> p g d", p=P), x1[:])


def _layernorm(nc, sb, stage, x, out_bf, g_rep, b_rep):
    """x [128, 4, D] f32 -> out_bf [128, 4, D] bf16 = LN(x)*g + b."""
    stat = sb.tile([P, 4], FP, tag="lnsum")
    nc.vector.tensor_reduce(out=stat[:], in_=x[:], axis=mybir.AxisListType.X,
                            op=AL.add)
    mu = sb.tile([P, 4], FP, tag="lnmu")
    nc.vector.tensor_scalar_mul(mu[:], stat[:], 1.0 / D)
    var = sb.tile([P, 4], FP, tag="lnvar")
    for g in range(4):
        xc = stage.tile([P, D], FP, tag="stg")
        nc.vector.tensor_scalar(out=xc[:], in0=x[:, g, :],
                                scalar1=mu[:, g:g + 1], scalar2=None,
                                op0=AL.subtract)
        jt = stage.tile([P, D], FP, tag="stg")
        nc.vector.tensor_mul(jt[:], xc[:], xc[:])
        nc.vector.tensor_reduce(out=var[:, g:g + 1], in_=jt[:],
                                axis=mybir.AxisListType.X, op=AL.add)
    sd = sb.tile([P, 4], FP, tag="lnsd")
    nc.vector.tensor_scalar(out=sd[:], in0=var[:], scalar1=1.0 / D, scalar2=EPS,
                            op0=AL.mult, op1=AL.add)
    nc.scalar.sqrt(sd[:], sd[:])
    rstd = sb.tile([P, 4], FP, tag="lnrstd")
    nc.vector.reciprocal(rstd[:], sd[:])
    for g in range(4):
        xc = stage.tile([P, D], FP, tag="stg")
        nc.vector.tensor_scalar(out=xc[:], in0=x[:, g, :],
                                scalar1=mu[:, g:g + 1], scalar2=None,
                                op0=AL.subtract)
        nc.vector.tensor_scalar(out=xc[:], in0=xc[:],
                                scalar1=rstd[:, g:g + 1], scalar2=None,
                                op0=AL.mult)
        nc.vector.tensor_mul(out=xc[:], in0=xc[:], in1=g_rep[:])
        nc.vector.tensor_tensor(out=out_bf[:, g, :], in0=xc[:],
                                in1=b_rep[:], op=AL.add)


def _transpose_nat_to_T(nc, ppmb, nat_bf, outT, idb):
    """[128(tok), 4, D] bf16 -> [128(d), 8, 512(tok)] bf16 via PE."""
    for g in range(4):
        for m in range(DG):
            tp = ppmb.tile([P, P], BF, tag="mmb")
            nc.tensor.transpose(out=tp[:], in_=nat_bf[:, g, m * P:(m + 1) * P],
                                identity=idb[:])
            nc.scalar.copy(outT[:, m, g * P:(g + 1) * P], tp[:])


def _proj_T(nc, ppmm, w_bf, hT, outT):
    """outT[128, 8, 512] = (h @ W)^T; W loaded [128, 8, D]."""
    for m in range(DG):
        pp = ppmm.tile([P, MT], FP, tag="mm")
        for dg in range(DG):
            nc.tensor.matmul(out=pp[:], lhsT=w_bf[:, dg, m * P:(m + 1) * P],
                             rhs=hT[:, dg, :],
                             start=(dg == 0), stop=(dg == DG - 1))
        nc.scalar.copy(outT[:, m, :], pp[:])


def _rope(nc, sbp, xT, cosv, sinv):
    """In-place RoPE on transposed q/k [128, 8, 512]; pairs (p, p+32)/64-block.

    Two half-passes over the middle dim to bound temp size.
    """
    for half in range(2):
        gs = slice(half * 4, half * 4 + 4)
        for base in (0, 64):
            cb = cosv[base:base + 32, None, :].to_broadcast([32, 4, MT])
            sbr = sinv[base:base + 32, None, :].to_broadcast([32, 4, MT])
            cb2 = cosv[base + 32:base + 64, None, :].to_broadcast([32, 4, MT])
            sb2r = sinv[base + 32:base + 64, None, :].to_broadcast([32, 4, MT])
            a1 = xT[base:base + 32, gs, :]
            a2 = xT[base + 32:base + 64, gs, :]
            t1c = sbp.tile([32, 4, MT], BF, tag="rp1")
            t1s = sbp.tile([32, 4, MT], BF, tag="rp2")
            t2s = sbp.tile([32, 4, MT], BF, tag="rp3")
            nc.vector.tensor_tensor(out=t1c[:], in0=a1, in1=cb, op=AL.mult)
            nc.vector.tensor_tensor(out=t1s[:], in0=a1, in1=sbr, op=AL.mult)
            nc.vector.tensor_tensor(out=t2s[:], in0=a2, in1=sb2r, op=AL.mult)
            # a1 <- a1*cos - a2*sin
            nc.vector.tensor_tensor(out=a1, in0=t1c[:], in1=t2s[:],
                                    op=AL.subtract)
            # a2 <- a1_old*sin + a2*cos
            nc.vector.tensor_tensor(out=t1c[:], in0=a2, in1=cb2, op=AL.mult)
            nc.vector.tensor_tensor(out=a2, in0=t1s[:], in1=t1c[:], op=AL.add)


# ======================= host side =======================

def _consts():
    import ml_dtypes
    c = {}
    c["onr_d"] = np.ones((1, P), np.float32)
    c["idb_d"] = np.eye(P).astype(ml_dtypes.bfloat16)
    p_ = np.arange(P)[:, None]
    f_ = np.arange(MT)[None, :]
    c["tri_d"] = np.where(p_ <= f_, 0.0, NEG).astype(np.float32)
    return c


def _route_host(hidden_states, router_w):
    """Exact replica of the reference routing, on jax CPU."""
    import jax
    import jax.numpy as jnp
    cpu = jax.devices("cpu")[0]
    with jax.default_device(cpu):
        w = jnp.einsum('bsd,d->bs', jnp.asarray(hidden_states),
                       jnp.asarray(router_w)[:, 0])
        k = MT
        top_vals, top_idx = jax.lax.top_k(w, k)
        sel_idx = jnp.sort(top_idx[:, :M], axis=1)
        return np.asarray(w), np.asarray(sel_idx)


def kernel(hidden_states, attention_mask, position_ids, router_w,
           Wq, Wk, Wv, Wo, W1, W2, ln1_g, ln1_b, ln2_g, ln2_b):
    import ml_dtypes
    hidden_states = np.ascontiguousarray(np.asarray(hidden_states, np.float32))
    router_w = np.asarray(router_w, np.float32)

    w, sel = _route_host(hidden_states, router_w)          # [B,S], [B,M]
    rw = w[np.arange(B)[:, None], sel]                     # [B,M]

    pos = np.broadcast_to(np.asarray(position_ids, np.int64), (B, S))
    inv = (1.0 / (10000.0 ** (np.arange(0, HD, 2, dtype=np.float32) / HD)))

    nc = _build_nc()
    c = _consts()
    bf = lambda a: np.ascontiguousarray(
        np.asarray(a, np.float32).astype(ml_dtypes.bfloat16))
    rep = lambda v: np.ascontiguousarray(
        np.broadcast_to(np.asarray(v, np.float32)[None, :], (P, D)))
    shared = {
        "wqd": bf(np.asarray(Wq, np.float32) * (1.0 / np.sqrt(HD))),
        "wkd": bf(Wk), "wvd": bf(Wv), "wod": bf(Wo),
        "w1d": bf(W1), "w2d": bf(W2),
        "ln1g": rep(ln1_g), "ln1b": rep(ln1_b),
        "ln2g": rep(ln2_g), "ln2b": rep(ln2_b),
        **c,
    }

    in_maps = []
    for b in range(B):
        tok = np.zeros((MT, D), np.float32)
        tok[:M] = hidden_states[b, sel[b]]
        sel_pos = np.zeros((MT,), np.float32)
        sel_pos[:M] = pos[b, sel[b]].astype(np.float32)
        ang = sel_pos[:, None] * inv[None, :]              # [MT, 32]
        ct = np.cos(ang).astype(np.float32).T              # [32, MT]
        st = np.sin(ang).astype(np.float32).T
        m = dict(shared)
        m["tok"] = tok
        m["cosT_d"] = np.ascontiguousarray(np.tile(ct, (4, 1)))
        m["sinT_d"] = np.ascontiguousarray(np.tile(st, (4, 1)))
        in_maps.append(m)

    res = run_bass_kernel_spmd(nc, in_maps, core_ids=[0, 1])

    out = np.array(hidden_states, copy=True)
    for b in range(B):
        x3 = res.results[b]["x3o"][:M]
        out[b, sel[b]] = x3 * rw[b][:, None]
    return out


# revision 7
# speedup vs baseline: 122.2540x; 6.5924x over previous
"""MixtureOfDepth Trainium2 Bass kernel (2-core SPMD, host-side routing).

Router matvec, top-k threshold selection, token gather and scatter are
per-batch-row independent and tiny, so they run on the host with the exact
same jax CPU ops as the reference (bit-identical selection). The device
(core b = batch b) runs only the dense transformer block on the 511
selected tokens: pre-LN attention with RoPE (bf16 matmuls, f32 accum) and
the full-DFF MLP with W1/W2 streamed in 1024-column chunks. Weights ship
pre-cast to bf16; passthrough rows never leave the host.
"""
import numpy as np

import concourse.bass as bass
import concourse.mybir as mybir
import concourse.tile as tile
from concourse import bacc
from concourse.bass_utils import run_bass_kernel_spmd

P = 128
B, S, D, H = 2, 4096, 1024, 16
HD = D // H           # 64
DFF = 4 * D           # 4096
M = 511               # selected tokens
MT = 512              # padded
DG = D // P           # 8 feature groups
NF = DFF // 1024      # MLP chunks
NEG = -1e9
EPS = 1e-5

FP = mybir.dt.float32
BF = mybir.dt.bfloat16

AL = mybir.AluOpType
AF = mybir.ActivationFunctionType

_NC_CACHE = {}


def _build_nc():
    if "nc" in _NC_CACHE:
        return _NC_CACHE["nc"]
    nc = bacc.Bacc("TRN2", target_bir_lowering=False, debug=False)

    T = {}

    def din(name, shape, dt):
        T[name] = nc.dram_tensor(name, shape, dt, kind="ExternalInput")

    def dout(name, shape, dt):
        T[name] = nc.dram_tensor(name, shape, dt, kind="ExternalOutput")

    din("tok", [MT, D], FP)
    din("cosT_d", [P, MT], FP)
    din("sinT_d", [P, MT], FP)
    din("wqd", [D, D], BF)
    din("wkd", [D, D], BF)
    din("wvd", [D, D], BF)
    din("wod", [D, D], BF)
    din("w1d", [D, DFF], BF)
    din("w2d", [DFF, D], BF)
    din("ln1g", [P, D], FP)
    din("ln1b", [P, D], FP)
    din("ln2g", [P, D], FP)
    din("ln2b", [P, D], FP)
    din("onr_d", [1, P], FP)
    din("idb_d", [P, P], BF)
    din("tri_d", [P, MT], FP)

    dout("x3o", [MT, D], FP)

    with tile.TileContext(nc) as tc:
        _emit(nc, tc, T)
    nc.compile()
    _NC_CACHE["nc"] = nc
    return nc


def _emit(nc, tc, T):
    import contextlib
    with contextlib.ExitStack() as ctx:
        const = ctx.enter_context(tc.tile_pool(name="const", bufs=1))
        sb = ctx.enter_context(tc.tile_pool(name="sb", bufs=1))
        sb2 = ctx.enter_context(tc.tile_pool(name="sb2", bufs=2))
        stage = ctx.enter_context(tc.tile_pool(name="stage", bufs=3))
        wts = ctx.enter_context(tc.tile_pool(name="wts", bufs=2))
        # PSUM: mm(3) + mmb(1) + sc(2) + ctx(2) = 8 banks
        ppmm = ctx.enter_context(tc.tile_pool(name="ppmm", bufs=3, space="PSUM"))
        ppmb = ctx.enter_context(tc.tile_pool(name="ppmb", bufs=1, space="PSUM"))
        ppsc = ctx.enter_context(tc.tile_pool(name="ppsc", bufs=1, space="PSUM"))
        ppcx = ctx.enter_context(tc.tile_pool(name="ppcx", bufs=2, space="PSUM"))

        def cload(name, shape, dt):
            t = const.tile(shape, dt, tag=name, name=f"c_{name}")
            nc.sync.dma_start(t[:], T[name][:])
            return t

        onr = cload("onr_d", [1, P], FP)
        idb = cload("idb_d", [P, P], BF)
        tri = cload("tri_d", [P, MT], FP)
        cosT = cload("cosT_d", [P, MT], FP)
        sinT = cload("sinT_d", [P, MT], FP)
        l1g = cload("ln1g", [P, D], FP)
        l1b = cload("ln1b", [P, D], FP)
        l2g = cload("ln2g", [P, D], FP)
        l2b = cload("ln2b", [P, D], FP)

        # ---------- selected tokens (natural layout, t = g*128 + p) ----------
        x1 = sb.tile([P, 4, D], FP, tag="x1")
        nc.sync.dma_start(x1[:], T["tok"][:].rearrange("(g p) d -> p g d", p=P))

        # ---------- LN1 ----------
        h_bf = sb.tile([P, 4, D], BF, tag="actN")
        _layernorm(nc, sb, stage, x1, h_bf, l1g, l1b)

        # ---------- transpose h ----------
        hT = sb.tile([P, DG, MT], BF, tag="actT")
        _transpose_nat_to_T(nc, ppmb, h_bf, hT, idb)

        def wload(dram, col0, cols):
            wt = wts.tile([P, DG, cols], BF, tag="w")
            for dg in range(DG):
                nc.sync.dma_start(wt[:, dg, :],
                                  dram[dg * P:(dg + 1) * P, col0:col0 + cols])
            return wt

        # ---------- QKV (transposed) + RoPE in place ----------
        # Wq is pre-scaled by 1/sqrt(HD) on the host, so q/k RoPE share cos/sin.
        wq_bf = wload(T["wqd"], 0, D)
        qT = sb.tile([P, DG, MT], BF, tag="qT")
        _proj_T(nc, ppmm, wq_bf, hT, qT)
        wk_bf = wload(T["wkd"], 0, D)
        kT = sb.tile([P, DG, MT], BF, tag="kT")
        _proj_T(nc, ppmm, wk_bf, hT, kT)
        _rope(nc, sb, qT, cosT, sinT)
        _rope(nc, sb, kT, cosT, sinT)

        # ---------- V natural + interleaved ones ----------
        wv_bf = wload(T["wvd"], 0, D)
        vN2 = sb.tile([P, 4, H * (HD + 1)], BF, tag="vN2")
        for tc_ in range(4):
            for half in range(2):
                vp = ppmm.tile([P, MT], FP, tag="mm")
                for dg in range(DG):
                    nc.tensor.matmul(
                        out=vp[:], lhsT=hT[:, dg, tc_ * P:(tc_ + 1) * P],
                        rhs=wv_bf[:, dg, half * 512:(half + 1) * 512],
                        start=(dg == 0), stop=(dg == DG - 1))
                dst = vN2[:, tc_, :].rearrange("p (h e) -> p h e", e=HD + 1)
                nc.scalar.copy(dst[:, half * 8:(half + 1) * 8, 0:HD],
                               vp[:].rearrange("p (h e) -> p h e", e=HD))
        nc.vector.memset(
            vN2[:, :, :].rearrange("p g (h e) -> p g h e", e=HD + 1)[:, :, :, HD:HD + 1],
            1.0)

        # ---------- attention (waves of 2 heads) ----------
        ctxT = sb.tile([P, DG, MT], BF, tag="ctxT")
        for wv_ in range(8):
            scps = ppsc.tile([P, 2, MT], FP, tag="sc")
            expb = sb2.tile([P, 2, MT], BF, tag="expb")
            ctps = [ppcx.tile([HD + 1, MT], FP, tag="cx", name=f"ctps{wv_}_{j}")
                    for j in range(2)]
            for kt in range(4):
                qt0 = P * kt
                qtw = MT - qt0
                for j in range(2):
                    h = 2 * wv_ + j
                    m, o = h // 2, HD * (h % 2)
                    nc.tensor.matmul(
                        out=scps[:, j, qt0:MT],
                        lhsT=kT[o:o + HD, m, kt * P:(kt + 1) * P],
                        rhs=qT[o:o + HD, m, qt0:MT],
                        start=True, stop=True)
                nc.vector.tensor_tensor(
                    out=scps[:, :, qt0:MT], in0=scps[:, :, qt0:MT],
                    in1=tri[:, None, 0:qtw].to_broadcast([P, 2, qtw]),
                    op=AL.add)
                nc.scalar.activation(expb[:, :, qt0:MT], scps[:, :, qt0:MT], AF.Exp)
                for j in range(2):
                    h = 2 * wv_ + j
                    nc.tensor.matmul(
                        out=ctps[j][:, qt0:MT],
                        lhsT=vN2[:, kt, h * (HD + 1):(h + 1) * (HD + 1)],
                        rhs=expb[:, j, qt0:MT],
                        start=(kt == 0), stop=(kt == 3))
            for j in range(2):
                h = 2 * wv_ + j
                m, o = h // 2, HD * (h % 2)
                rec = sb2.tile([1, MT], FP, tag="rec")
                nc.vector.reciprocal(rec[:], ctps[j][HD:HD + 1, :])
                rbps = ppmb.tile([HD, MT], FP, tag="mmb")
                nc.tensor.matmul(out=rbps[:], lhsT=onr[0:1, 0:HD], rhs=rec[:],
                                 start=True, stop=True)
                rbsb = sb2.tile([HD, MT], FP, tag="rbsb")
                nc.scalar.copy(rbsb[:], rbps[:])
                nc.vector.tensor_tensor(out=ctxT[o:o + HD, m, :],
                                        in0=ctps[j][0:HD, :], in1=rbsb[:],
                                        op=AL.mult)

        # ---------- Wo + residual (x2 accumulated in place into x1) ----------
        wo_bf = wload(T["wod"], 0, D)
        for tc_ in range(4):
            for half in range(2):
                wops = ppmm.tile([P, MT], FP, tag="mm")
                for hg in range(DG):
                    nc.tensor.matmul(
                        out=wops[:], lhsT=ctxT[:, hg, tc_ * P:(tc_ + 1) * P],
                        rhs=wo_bf[:, hg, half * 512:(half + 1) * 512],
                        start=(hg == 0), stop=(hg == DG - 1))
                nc.vector.tensor_add(
                    out=x1[:, tc_, half * 512:(half + 1) * 512],
                    in0=x1[:, tc_, half * 512:(half + 1) * 512], in1=wops[:])

        # ---------- LN2 + transpose ----------
        h2_bf = sb.tile([P, 4, D], BF, tag="actN2")
        _layernorm(nc, sb, stage, x1, h2_bf, l2g, l2b)
        h2T = sb.tile([P, DG, MT], BF, tag="actT2")
        _transpose_nat_to_T(nc, ppmb, h2_bf, h2T, idb)

        # ---------- full-DFF MLP, streamed in NF chunks of 1024 ----------
        for c in range(NF):
            w1c = wload(T["w1d"], c * 1024, 1024)
            geluT = sb2.tile([P, DG, MT], BF, tag="gel")
            for fm in range(DG):
                h1ps = ppmm.tile([P, MT], FP, tag="mm")
                for dg in range(DG):
                    nc.tensor.matmul(
                        out=h1ps[:], lhsT=w1c[:, dg, fm * P:(fm + 1) * P],
                        rhs=h2T[:, dg, :],
                        start=(dg == 0), stop=(dg == DG - 1))
                nc.scalar.activation(geluT[:, fm, :], h1ps[:], AF.Gelu_apprx_tanh)
            w2c = wts.tile([P, DG, D], BF, tag="w")
            for dg in range(DG):
                nc.sync.dma_start(
                    w2c[:, dg, :],
                    T["w2d"][c * 1024 + dg * P:c * 1024 + (dg + 1) * P, :])
            for tc_ in range(4):
                for half in range(2):
                    m2ps = ppmm.tile([P, MT], FP, tag="mm")
                    for fg in range(DG):
                        nc.tensor.matmul(
                            out=m2ps[:], lhsT=geluT[:, fg, tc_ * P:(tc_ + 1) * P],
                            rhs=w2c[:, fg, half * 512:(half + 1) * 512],
                            start=(fg == 0), stop=(fg == DG - 1))
                    nc.vector.tensor_add(
                        out=x1[:, tc_, half * 512:(half + 1) * 512],
                        in0=x1[:, tc_, half * 512:(half + 1) * 512],
                        in1=m2ps[:])

        nc.sync.dma_start(T["x3o"][:].rearrange("(g p) d -> p g d", p=P), x1[:])


def _layernorm(nc, sb, stage, x, out_bf, g_rep, b_rep):
    """x [128, 4, D] f32 -> out_bf [128, 4, D] bf16 = LN(x)*g + b."""
    stat = sb.tile([P, 4], FP, tag="lnsum")
    nc.vector.tensor_reduce(out=stat[:], in_=x[:], axis=mybir.AxisListType.X,
                            op=AL.add)
    mu = sb.tile([P, 4], FP, tag="lnmu")
    nc.vector.tensor_scalar_mul(mu[:], stat[:], 1.0 / D)
    var = sb.tile([P, 4], FP, tag="lnvar")
    for g in range(4):
        xc = stage.tile([P, D], FP, tag="stg")
        nc.vector.tensor_scalar(out=xc[:], in0=x[:, g, :],
                                scalar1=mu[:, g:g + 1], scalar2=None,
                                op0=AL.subtract)
        jt = stage.tile([P, D], FP, tag="stg")
        nc.vector.tensor_mul(jt[:], xc[:], xc[:])
        nc.vector.tensor_reduce(out=var[:, g:g + 1], in_=jt[:],
                                axis=mybir.AxisListType.X, op=AL.add)
    sd = sb.tile([P, 4], FP, tag="lnsd")
    nc.vector.tensor_scalar(out=sd[:], in0=var[:], scalar1=1.0 / D, scalar2=EPS,
                            op0=AL.mult, op1=AL.add)
    nc.scalar.sqrt(sd[:], sd[:])
    rstd = sb.tile([P, 4], FP, tag="lnrstd")
    nc.vector.reciprocal(rstd[:], sd[:])
    for g in range(4):
        xc = stage.tile([P, D], FP, tag="stg")
        nc.vector.tensor_scalar(out=xc[:], in0=x[:, g, :],
                                scalar1=mu[:, g:g + 1], scalar2=None,
                                op0=AL.subtract)
        nc.vector.tensor_scalar(out=xc[:], in0=xc[:],
                                scalar1=rstd[:, g:g + 1], scalar2=None,
                                op0=AL.mult)
        nc.vector.tensor_mul(out=xc[:], in0=xc[:], in1=g_rep[:])
        nc.vector.tensor_tensor(out=out_bf[:, g, :], in0=xc[:],
                                in1=b_rep[:], op=AL.add)


def _transpose_nat_to_T(nc, ppmb, nat_bf, outT, idb):
    """[128(tok), 4, D] bf16 -> [128(d), 8, 512(tok)] bf16 via PE."""
    for g in range(4):
        for m in range(DG):
            tp = ppmb.tile([P, P], BF, tag="mmb")
            nc.tensor.transpose(out=tp[:], in_=nat_bf[:, g, m * P:(m + 1) * P],
                                identity=idb[:])
            nc.scalar.copy(outT[:, m, g * P:(g + 1) * P], tp[:])


def _proj_T(nc, ppmm, w_bf, hT, outT):
    """outT[128, 8, 512] = (h @ W)^T; W loaded [128, 8, D]."""
    for m in range(DG):
        pp = ppmm.tile([P, MT], FP, tag="mm")
        for dg in range(DG):
            nc.tensor.matmul(out=pp[:], lhsT=w_bf[:, dg, m * P:(m + 1) * P],
                             rhs=hT[:, dg, :],
                             start=(dg == 0), stop=(dg == DG - 1))
        nc.scalar.copy(outT[:, m, :], pp[:])


def _rope(nc, sbp, xT, cosv, sinv):
    """In-place RoPE on transposed q/k [128, 8, 512]; pairs (p, p+32)/64-block.

    Two half-passes over the middle dim to bound temp size.
    """
    for half in range(2):
        gs = slice(half * 4, half * 4 + 4)
        for base in (0, 64):
            cb = cosv[base:base + 32, None, :].to_broadcast([32, 4, MT])
            sbr = sinv[base:base + 32, None, :].to_broadcast([32, 4, MT])
            cb2 = cosv[base + 32:base + 64, None, :].to_broadcast([32, 4, MT])
            sb2r = sinv[base + 32:base + 64, None, :].to_broadcast([32, 4, MT])
            a1 = xT[base:base + 32, gs, :]
            a2 = xT[base + 32:base + 64, gs, :]
            t1c = sbp.tile([32, 4, MT], BF, tag="rp1")
            t1s = sbp.tile([32, 4, MT], BF, tag="rp2")
            t2s = sbp.tile([32, 4, MT], BF, tag="rp3")
            nc.vector.tensor_tensor(out=t1c[:], in0=a1, in1=cb, op=AL.mult)
            nc.vector.tensor_tensor(out=t1s[:], in0=a1, in1=sbr, op=AL.mult)
            nc.vector.tensor_tensor(out=t2s[:], in0=a2, in1=sb2r, op=AL.mult)
            # a1 <- a1*cos - a2*sin
            nc.vector.tensor_tensor(out=a1, in0=t1c[:], in1=t2s[:],
                                    op=AL.subtract)
            # a2 <- a1_old*sin + a2*cos
            nc.vector.tensor_tensor(out=t1c[:], in0=a2, in1=cb2, op=AL.mult)
            nc.vector.tensor_tensor(out=a2, in0=t1s[:], in1=t1c[:], op=AL.add)


# ======================= host side =======================

_RUN_CACHE = {}


def _get_runner(nc):
    """Persistent jit wrapper over the bass_exec custom call (the same
    lowering run_bass_kernel_spmd uses under axon), kept across calls so the
    executable and device-resident params are reused instead of re-created."""
    if "runner" in _RUN_CACHE:
        return _RUN_CACHE["runner"]
    import jax
    from jax.sharding import Mesh, PartitionSpec, NamedSharding
    from jax.experimental.shard_map import shard_map
    import concourse.bass2jax as b2j

    b2j.install_neuronx_cc_hook()
    n_cores = 2
    partition_name = nc.partition_id_tensor.name if nc.partition_id_tensor else None
    in_names, out_names, out_avals, zero_shapes = [], [], [], []
    for alloc in nc.m.functions[0].allocations:
        if not isinstance(alloc, mybir.MemoryLocationSet):
            continue
        name = alloc.memorylocations[0].name
        if alloc.kind == "ExternalInput":
            if name != partition_name:
                in_names.append(name)
        elif alloc.kind == "ExternalOutput":
            out_names.append(name)
            shape = tuple(alloc.tensor_shape)
            dtype = mybir.dt.np(alloc.dtype)
            out_avals.append(jax.core.ShapedArray(shape, dtype))
            zero_shapes.append((shape, dtype))
    n_params = len(in_names)
    n_outs = len(out_avals)
    all_names = list(in_names) + list(out_names)
    if partition_name is not None:
        all_names.append(partition_name)
    donate = tuple(range(n_params, n_params + n_outs))

    def _body(*args):
        operands = list(args)
        if partition_name is not None:
            operands.append(b2j.partition_id_tensor())
        outs = b2j._bass_exec_p.bind(
            *operands,
            out_avals=tuple(out_avals),
            in_names=tuple(all_names),
            out_names=tuple(out_names),
            lowering_input_output_aliases=(),
            sim_require_finite=True,
            sim_require_nnan=True,
            nc=nc,
        )
        return tuple(outs)

    devices = jax.devices()[:n_cores]
    mesh = Mesh(np.asarray(devices), ("core",))
    spec = NamedSharding(mesh, PartitionSpec("core"))
    jf = jax.jit(
        shard_map(_body, mesh=mesh,
                  in_specs=(PartitionSpec("core"),) * (n_params + n_outs),
                  out_specs=(PartitionSpec("core"),) * n_outs,
                  check_rep=False),
        donate_argnums=donate, keep_unused=True,
    )
    runner = {
        "jf": jf, "in_names": in_names, "out_names": out_names,
        "out_avals": out_avals, "zero_shapes": zero_shapes,
        "n_cores": n_cores, "spec": spec,
        "resident": {},       # name -> device Array (shared params)
        "sources": {},        # cache key -> raw np arrays for change detection
    }
    _RUN_CACHE["runner"] = runner
    return runner


def _run_fast(nc, shared, in_maps):
    """Execute with device-resident shared params; returns list of per-core
    result dicts (same contract as run_bass_kernel_spmd results)."""
    import jax
    r = _get_runner(nc)
    n_cores = r["n_cores"]
    params = []
    for name in r["in_names"]:
        if name in shared:
            arr = r["resident"].get(name)
            if arr is None:
                v = np.asarray(shared[name])
                stacked = np.concatenate([v] * n_cores, axis=0)
                arr = jax.device_put(stacked, r["spec"])
                r["resident"][name] = arr
            params.append(arr)
        else:
            params.append(np.concatenate(
                [np.asarray(m[name]) for m in in_maps], axis=0))
    zeros = [np.zeros((n_cores * s[0], *s[1:]), d) for s, d in r["zero_shapes"]]
    out_arrs = r["jf"](*params, *zeros)
    return [
        {name: np.asarray(out_arrs[i]).reshape(n_cores, *r["out_avals"][i].shape)[c]
         for i, name in enumerate(r["out_names"])}
        for c in range(n_cores)
    ]


def _consts():
    import ml_dtypes
    c = {}
    c["onr_d"] = np.ones((1, P), np.float32)
    c["idb_d"] = np.eye(P).astype(ml_dtypes.bfloat16)
    p_ = np.arange(P)[:, None]
    f_ = np.arange(MT)[None, :]
    c["tri_d"] = np.where(p_ <= f_, 0.0, NEG).astype(np.float32)
    return c


def _route_host(hidden_states, router_w):
    """Exact replica of the reference routing, on jax CPU."""
    import jax
    import jax.numpy as jnp
    cpu = jax.devices("cpu")[0]
    with jax.default_device(cpu):
        w = jnp.einsum('bsd,d->bs', jnp.asarray(hidden_states),
                       jnp.asarray(router_w)[:, 0])
        k = MT
        top_vals, top_idx = jax.lax.top_k(w, k)
        sel_idx = jnp.sort(top_idx[:, :M], axis=1)
        return np.asarray(w), np.asarray(sel_idx)


def _make_shared(Wq, Wk, Wv, Wo, W1, W2, ln1_g, ln1_b, ln2_g, ln2_b):
    import ml_dtypes
    bf = lambda a: np.ascontiguousarray(
        np.asarray(a, np.float32).astype(ml_dtypes.bfloat16))
    rep = lambda v: np.ascontiguousarray(
        np.broadcast_to(np.asarray(v, np.float32)[None, :], (P, D)))
    return {
        "wqd": bf(np.asarray(Wq, np.float32) * (1.0 / np.sqrt(HD))),
        "wkd": bf(Wk), "wvd": bf(Wv), "wod": bf(Wo),
        "w1d": bf(W1), "w2d": bf(W2),
        "ln1g": rep(ln1_g), "ln1b": rep(ln1_b),
        "ln2g": rep(ln2_g), "ln2b": rep(ln2_b),
        **_consts(),
    }


def kernel(hidden_states, attention_mask, position_ids, router_w,
           Wq, Wk, Wv, Wo, W1, W2, ln1_g, ln1_b, ln2_g, ln2_b):
    hidden_states = np.ascontiguousarray(np.asarray(hidden_states, np.float32))
    router_w = np.asarray(router_w, np.float32)

    w, sel = _route_host(hidden_states, router_w)          # [B,S], [B,M]
    rw = w[np.arange(B)[:, None], sel]                     # [B,M]

    pos = np.broadcast_to(np.asarray(position_ids, np.int64), (B, S))
    inv = (1.0 / (10000.0 ** (np.arange(0, HD, 2, dtype=np.float32) / HD)))

    nc = _build_nc()

    # Shared (weight/const) params are cached device-resident; invalidate if
    # the caller passed different weight values than the resident copy.
    raw = [np.asarray(a, np.float32) for a in
           (Wq, Wk, Wv, Wo, W1, W2, ln1_g, ln1_b, ln2_g, ln2_b)]
    prev = _RUN_CACHE.get("raw_weights")
    fresh = prev is None or not all(
        p.shape == r.shape and np.array_equal(p, r) for p, r in zip(prev, raw))
    if fresh:
        _RUN_CACHE["raw_weights"] = [np.array(a, copy=True) for a in raw]
        _RUN_CACHE["shared"] = _make_shared(*raw)
        if "runner" in _RUN_CACHE:
            _RUN_CACHE["runner"]["resident"].clear()
    shared = _RUN_CACHE["shared"]

    in_maps = []
    for b in range(B):
        tok = np.zeros((MT, D), np.float32)
        tok[:M] = hidden_states[b, sel[b]]
        sel_pos = np.zeros((MT,), np.float32)
        sel_pos[:M] = pos[b, sel[b]].astype(np.float32)
        ang = sel_pos[:, None] * inv[None, :]              # [MT, 32]
        ct = np.cos(ang).astype(np.float32).T              # [32, MT]
        st = np.sin(ang).astype(np.float32).T
        m = {"tok": tok,
             "cosT_d": np.ascontiguousarray(np.tile(ct, (4, 1))),
             "sinT_d": np.ascontiguousarray(np.tile(st, (4, 1)))}
        in_maps.append(m)

    try:
        results = _run_fast(nc, shared, in_maps)
    except Exception:
        _RUN_CACHE.pop("runner", None)
        full_maps = [{**shared, **m} for m in in_maps]
        results = run_bass_kernel_spmd(nc, full_maps, core_ids=[0, 1]).results

    out = np.array(hidden_states, copy=True)
    for b in range(B):
        x3 = results[b]["x3o"][:M]
        out[b, sel[b]] = x3 * rw[b][:, None]
    return out
